# revision 1
# baseline (speedup 1.0000x reference)
"""Trainium2 Bass kernel for nn_DEC_26139170963600 (vq_codebook).

Reference computation:
  4x strided conv1d (stride 2, VALID) with LeakyReLU(0.1) between layers,
  flatten -> soft VQ assignment over 64 centers:
      d2 = ||z||^2 + ||c||^2 - 2 z.c
      q  = (1/(1+d2)) row-normalized            (alpha=1 -> exponent is 1)

Sharding: data-parallel over batch N=256 across 8 cores (32 samples/core).
Weights / centers replicated. No cross-device communication.

Per-core kernel design:
  - x in SBUF as (C=128 partitions, n*L) bf16, sample-major free dim.
  - conv layer = K tap-matmuls accumulated in PSUM:
        out[o, l] += W[o,:,k]^T . h[:, 2l+k]
    lhsT = W transposed to (i, o) per tap; rhs = strided slice of h.
    Later layers batch G samples per matmul (3D rhs AP) to keep the
    moving-operand free dim near 512 and amortize LDWEIGHTS.
  - PSUM eviction fuses bias + LeakyReLU: relu(y+b) - relu(-0.1(y+b)) as two
    ScalarE ops + one DVE subtract (exact; HW ACT Lrelu is broken here: it
    returns ~0.01x on negatives, micro-tested max rel err 0.9).
  - Distance: 59 bf16 matmuls accumulate -2 z.c into PSUM (32n x 64j);
    ||z||^2 via per-group DVE square+reduce (overlapped with conv4) then an
    fp32 matmul against a ones column; 1 + ||c||^2 comes in as a
    host-precomputed (32,64) fp32 tile (exact, avoids partition broadcast).
  - q = reciprocal(1+d2) row-normalized on DVE (DVE reciprocal is exact
    iterative divide), DMA out as fp32.
  - PE pre-warm: 44 dummy matmuls during the w1/x DMA lead-in so HAM
    un-throttles (1.2 -> 2.4 GHz) before real conv work arrives.

Measured (8 axon trn2 cores): max rel err 1.6e-4 vs fp32 reference;
~165-175 us/core steady-state vs ~157 us bf16 PE roofline (12.2 GFLOP/core
at 78.6 TF/s). fp16 would halve rounding error but hard-faults the device
(NRT_EXEC_UNIT_UNRECOVERABLE) - do not use.
"""

import os
import sys

import numpy as np
import ml_dtypes

for _p in ("/opt/trn_rl_repo",):
    if _p not in sys.path and os.path.isdir(_p):
        sys.path.insert(0, _p)

import concourse.bacc as bacc  # noqa: E402
import concourse.mybir as mybir  # noqa: E402
import concourse.tile as tile  # noqa: E402
from concourse import bass_utils  # noqa: E402

HDT = mybir.dt.bfloat16  # NOTE: fp16 matmuls hard-fault trn2 here (NRT_EXEC_UNIT_UNRECOVERABLE)
F32 = mybir.dt.float32
AF = mybir.ActivationFunctionType
OP = mybir.AluOpType

N_CORES = 8
NS = 32          # samples per core
C = 128          # channels
KCENT = 64       # number of centers
LFIN = 59        # final length
D = C * LFIN     # 7552

# (K, L_in, L_out, G samples per matmul)
CFG = [
    (15, 1024, 505, 1),
    (12, 505, 247, 2),
    (7, 247, 121, 4),
    (4, 121, 59, 8),
]

USE_LRELU = False  # HW Lrelu is BROKEN here (negatives ~0.01x, not alpha*x;
# micro-tested max rel err 0.9). relu(y)-relu(-0.1y) pair is exact.

_BUILt = {}


def _build_program(n_repeat=1):
    """Build + compile the per-core Bass program (same program on all cores).

    n_repeat > 1 unrolls the full per-inference body that many times inside
    one NEFF (constants loaded once) — used only for slope timing in bench.py.
    """
    nc = bacc.Bacc("TRN2", target_bir_lowering=False, debug=False)

    # ---- DRAM I/O ----
    x_d = nc.dram_tensor("x", (C, NS, 1024), HDT, kind="ExternalInput")
    w_d = [
        nc.dram_tensor(f"w{i+1}", (C, CFG[i][0] * C), HDT, kind="ExternalInput")
        for i in range(4)
    ]
    # bias pack: cols 0-3 = b1..b4; cols 4-6 = -0.1*b1..b3; col 7 = ones
    bp_d = nc.dram_tensor("bp", (C, 8), F32, kind="ExternalInput")
    cr_d = nc.dram_tensor("cr", (C, LFIN * KCENT), HDT, kind="ExternalInput")
    cnb_d = nc.dram_tensor("cnb", (NS, KCENT), F32, kind="ExternalInput")
    q_d = nc.dram_tensor("q", (NS, KCENT), F32, kind="ExternalOutput")

    with tile.TileContext(nc) as tc:
        with (
            tc.tile_pool(name="consts", bufs=1) as cpool,
            tc.tile_pool(name="xp", bufs=8) as xpool,
            tc.tile_pool(name="hp", bufs=1) as hpool,
            tc.tile_pool(name="sp", bufs=2) as spool,
            tc.tile_pool(name="small", bufs=1) as mpool,
            tc.tile_pool(name="psA", bufs=6, space="PSUM") as psA,
            tc.tile_pool(name="psZ", bufs=1, space="PSUM") as psZ,
            tc.tile_pool(name="psD", bufs=1, space="PSUM") as psD,
        ):
            # ---- const tiles (DMA'd inside the body, w1 first) ----
            wt = [
                cpool.tile([C, CFG[i][0] * C], HDT, tag=f"w{i}", name=f"wt{i}")
                for i in range(4)
            ]
            bp = cpool.tile([C, 8], F32, tag="bp")
            cr = cpool.tile([C, LFIN * KCENT], HDT, tag="cr")
            cnb = cpool.tile([NS, KCENT], F32, tag="cnb")

            for _rep in range(n_repeat):
                _body_once(nc, tc, x_d, q_d, w_d, bp_d, cr_d, cnb_d, wt, bp,
                           cr, cnb, xpool, hpool, spool, mpool, psA, psZ, psD,
                           load_consts=(_rep == 0))

    nc.compile()
    return nc


def _body_once(nc, tc, x_d, q_d, w_d, bp_d, cr_d, cnb_d, wt, bp, cr, cnb,
               xpool, hpool, spool, mpool, psA, psZ, psD, load_consts=True):
            # ---- Two HWDGE rings: x chunks stream on the SP ring while all
            # constants go on the ACT ring, so w1 arrives concurrently with
            # x0 and conv1 starts ~2us sooner ----
            if load_consts:
                nc.scalar.dma_start(wt[0][:], w_d[0].ap())
                nc.scalar.dma_start(bp[:], bp_d.ap())
            xch = []
            for g in range(16):
                t = xpool.tile([C, 2 * 1024], HDT, tag="x", name=f"xch{g}")
                src = x_d.ap()[:, 2 * g : 2 * g + 2, :].rearrange("p a b -> p (a b)")
                nc.sync.dma_start(t[:], src)
                xch.append(t)
            if load_consts:
                for i in range(1, 4):
                    nc.scalar.dma_start(wt[i][:], w_d[i].ap())
                nc.scalar.dma_start(cr[:], cr_d.ap())
                nc.scalar.dma_start(cnb[:], cnb_d.ap())

                # ---- PE pre-warm: HAM un-throttles (1.2 -> 2.4 GHz) after
                # ~3.4us of sustained activity; burn the w1/x0 DMA lead-in on
                # dummy matmuls over a zeroed scratch so conv1 starts warm ----
                # K=1 contraction: streams 128 cols per dummy (same PE
                # busy-ness for HAM) but the scratch memset is one partition
                wsrc = spool.tile([1, 128], HDT, tag="warm", name="warm")
                nc.gpsimd.memset(wsrc[:], 0.0)
                wps = psA.tile([C, 128], F32, tag="ps", name="warmps")
                for _w in range(44):
                    nc.tensor.matmul(
                        wps[:], wsrc[:], wsrc[:], start=(_w == 0), stop=(_w == 43)
                    )

            # ---- conv stack ----
            h_tiles = []
            for li, (K, Lin, Lout, G) in enumerate(CFG):
                hdst = hpool.tile([C, NS * Lout], HDT, tag=f"h{li}")
                if li > 0:
                    hsrc3 = h_tiles[li - 1][:].rearrange("p (n l) -> p n l", n=NS)
                for g0 in range(0, NS, G):
                    ps = psA.tile([C, G * Lout], F32, tag="ps")
                    for k in range(K):
                        lhsT = wt[li][:, k * C : (k + 1) * C]
                        stop_idx = k + 2 * (Lout - 1) + 1
                        if li == 0:
                            x3 = xch[g0 // 2][:].rearrange("p (a b) -> p a b", a=2)
                            rhs = x3[:, g0 % 2 : g0 % 2 + 1, k : stop_idx : 2]
                        else:
                            rhs = hsrc3[:, g0 : g0 + G, k : stop_idx : 2]
                        nc.tensor.matmul(
                            ps[:], lhsT, rhs, start=(k == 0), stop=(k == K - 1)
                        )
                    dsl = hdst[:, g0 * Lout : (g0 + G) * Lout]
                    bias = bp[:, li : li + 1]
                    if li < 3:
                        if USE_LRELU:
                            nc.scalar.activation(
                                dsl, ps[:], AF.Lrelu, bias=bias, scale=1.0, alpha=0.1
                            )
                        else:
                            a = spool.tile([C, G * Lout], HDT, tag="a")
                            b2 = spool.tile([C, G * Lout], HDT, tag="b")
                            nbias = bp[:, 4 + li : 5 + li]
                            nc.scalar.activation(
                                a[:], ps[:], AF.Relu, bias=bias, scale=1.0
                            )
                            nc.scalar.activation(
                                b2[:], ps[:], AF.Relu, bias=nbias, scale=-0.1
                            )
                            nc.vector.tensor_tensor(dsl, a[:], b2[:], op=OP.subtract)
                    else:
                        nc.scalar.activation(
                            dsl, ps[:], AF.Identity, bias=bias, scale=1.0
                        )
                        # ||z||^2 partials per group, overlapped with the
                        # remaining conv4 PE work (shortens the tail)
                        if g0 == 0:
                            zsq = hpool.tile(
                                [C, NS * LFIN], F32, tag="zsq", name="zsq"
                            )
                            part = mpool.tile([C, NS], F32, tag="part", name="part")
                        zsl = zsq[:, g0 * LFIN : (g0 + G) * LFIN]
                        nc.vector.tensor_tensor(zsl, dsl, dsl, op=OP.mult)
                        nc.vector.tensor_reduce(
                            part[:, g0 : g0 + G],
                            zsl.rearrange("p (n l) -> p n l", n=G),
                            axis=mybir.AxisListType.X,
                            op=OP.add,
                        )
                h_tiles.append(hdst)

            zb = h_tiles[3]  # (128, 32*59) bf16, sample-major

            # ---- ||z||^2 per sample (partials already in `part`) ----
            zn_ps = psZ.tile([NS, 1], F32, tag="zn")
            ones = bp[:, 7:8]
            nc.tensor.matmul(zn_ps[:], part[:], ones, start=True, stop=True)
            zn1 = mpool.tile([NS, 1], F32, tag="zn1")
            nc.scalar.copy(zn1[:], zn_ps[:])

            # ---- -2 z.c accumulated over 59 position-chunks ----
            d_ps = psD.tile([NS, KCENT], F32, tag="d")
            for l in range(LFIN):
                lhsT = zb[:, l : l + LFIN * (NS - 1) + 1 : LFIN]  # (128, 32)
                rhs = cr[:, l * KCENT : (l + 1) * KCENT]  # (128, 64)
                nc.tensor.matmul(
                    d_ps[:], lhsT, rhs, start=(l == 0), stop=(l == LFIN - 1)
                )

            # ---- q = normalize(1/(1+d2)) ----
            t1 = mpool.tile([NS, KCENT], F32, tag="t1")
            nc.vector.tensor_scalar_add(t1[:], d_ps[:], zn1[:])
            nc.vector.tensor_tensor(t1[:], t1[:], cnb[:], op=OP.add)
            qn = mpool.tile([NS, KCENT], F32, tag="qn")
            nc.vector.reciprocal(qn[:], t1[:])
            rs = mpool.tile([NS, 1], F32, tag="rs")
            nc.vector.tensor_reduce(
                rs[:], qn[:], axis=mybir.AxisListType.X, op=OP.add
            )
            rr = mpool.tile([NS, 1], F32, tag="rr")
            nc.vector.reciprocal(rr[:], rs[:])
            nc.vector.tensor_scalar_mul(qn[:], qn[:], rr[:])
            nc.sync.dma_start(q_d.ap(), qn[:])


def _get_program(n_repeat=1):
    if n_repeat not in _BUILt:
        _BUILt[n_repeat] = _build_program(n_repeat)
    return _BUILt[n_repeat]


def _prep_inputs(x, w1, b1, w2, b2, w3, b3, w4, b4, centers):
    """Host-side prep: dtype casts, weight transposes, per-core sharding."""
    ws = [w1, w2, w3, w4]
    bs = [b1, b2, b3, b4]

    const_map = {}
    for i, w in enumerate(ws):
        K = CFG[i][0]
        # (O, I, K) -> (I, K, O) -> (128, K*128); lhsT tap k = [:, k*128:(k+1)*128]
        const_map[f"w{i+1}"] = np.ascontiguousarray(
            np.asarray(w, np.float32).transpose(1, 2, 0).reshape(C, K * C)
        ).astype(ml_dtypes.bfloat16)

    bp = np.zeros((C, 8), np.float32)
    for i, b in enumerate(bs):
        bp[:, i] = np.asarray(b, np.float32)
    for i in range(3):
        bp[:, 4 + i] = -0.1 * np.asarray(bs[i], np.float32)
    bp[:, 7] = 1.0
    const_map["bp"] = bp

    cent = np.asarray(centers, np.float32)
    # cr[c, l*64 + j] = -2 * centers[j, c*59 + l]
    const_map["cr"] = np.ascontiguousarray(
        (-2.0 * cent).reshape(KCENT, C, LFIN).transpose(1, 2, 0).reshape(C, LFIN * KCENT)
    ).astype(ml_dtypes.bfloat16)
    cn = 1.0 + (cent.astype(np.float64) ** 2).sum(axis=1)  # (64,)
    const_map["cnb"] = np.broadcast_to(
        cn.astype(np.float32)[None, :], (NS, KCENT)
    ).copy()

    xf = np.asarray(x, np.float32)
    in_maps = []
    for c in range(N_CORES):
        shard = xf[c * NS : (c + 1) * NS]  # (32, 128, 1024)
        xc = np.ascontiguousarray(shard.transpose(1, 0, 2)).astype(ml_dtypes.bfloat16)  # (128,32,1024)
        in_maps.append({"x": xc, **const_map})
    return in_maps


def _ensure_devices():
    """Absorb wedged-device attach faults with a tiny op before the real run.

    A previous process can leave a NeuronCore wedged
    (NRT_EXEC_UNIT_UNRECOVERABLE); the first attach after a wedge fails and
    triggers a reset that completes within ~60 s.
    """
    import time

    import jax
    import jax.numpy as jnp

    for attempt in range(3):
        try:
            outs = [jax.device_put(jnp.zeros((8,)), d) + 1.0 for d in jax.devices()]
            jax.block_until_ready(outs)
            return
        except Exception:  # noqa: BLE001 - device fault; wait out the reset
            if attempt == 2:
                raise
            time.sleep(60)


def run(trace=False, **inputs):
    """Run the kernel; returns (q_full, BassKernelResults).

    Retries on device-unrecoverable faults (see _ensure_devices).
    """
    import time

    _ensure_devices()
    nc = _get_program()
    in_maps = _prep_inputs(**inputs)
    last_err = None
    for attempt in range(3):
        try:
            res = bass_utils.run_bass_kernel_spmd(
                nc, in_maps, core_ids=list(range(N_CORES)), trace=trace
            )
            break
        except Exception as e:  # noqa: BLE001 - device fault, wait + retry
            last_err = e
            if "UNAVAILABLE" not in str(e) and "unrecoverable" not in str(e).lower():
                raise
            time.sleep(60)
    else:
        raise last_err
    q = np.concatenate([res.results[c]["q"] for c in range(N_CORES)], axis=0)
    return np.ascontiguousarray(q.astype(np.float32)), res


def kernel(**inputs) -> np.ndarray:
    q, _ = run(trace=False, **inputs)
    return q



# revision 28
# speedup vs baseline: 1503.1344x; 1503.1344x over previous
"""Trainium2 Bass kernel for nn_DEC_26139170963600 (vq_codebook).

Reference computation:
  4x strided conv1d (stride 2, VALID) with LeakyReLU(0.1) between layers,
  flatten -> soft VQ assignment over 64 centers:
      d2 = ||z||^2 + ||c||^2 - 2 z.c
      q  = (1/(1+d2)) row-normalized            (alpha=1 -> exponent is 1)

Sharding: data-parallel over batch N=256 across 8 cores (32 samples/core).
Weights / centers replicated. No cross-device communication.

Per-core kernel design (fp8 DoubleRow conv stack):
  - x and all conv weights quantized host-side to fp8e4 (TRN E4M3, max 240).
    Weights are pre-scaled by a per-layer power of two (up to ~224 max mag)
    to clear e4m3 subnormals; the inverse scale rides the PSUM eviction.
  - conv layer = K/2 tap-pair matmuls in MatmulPerfMode.DoubleRow, which
    virtualizes the PE to a 256-deep contraction (2 fp8 weights/cell):
        out[o, l] += sum_i W[o,:,k+i]^T . h[:, 2l+k+i],  i in {0,1}
    lhsT = (128, 2, 128) tap-pair slice of the packed weights; rhs is the
    contiguous slice h[k : k+2*Lout] rearranged "(l two) -> two l" (the
    stride-2 conv makes tap pairs adjacent in memory). Odd K padded with a
    zero tap (conv1 15->16, conv3 7->8; h2 rows are stride-248-padded so
    the zero tap's read stays in-bounds).
  - PSUM eviction splits LeakyReLU over three engines so no engine exceeds
    the PE's busy time:  ACT: A = ps*2^-k + b (PSUM->SBUF bf16);
    Pool: t = 0.1*A;  DVE: h' = max(A, t) -> fp8 (exact lrelu: 0.1<1).
  - conv4 evicts bf16 z (no lrelu); distance stays bf16: 59 matmuls
    accumulate -2 z.c into PSUM (32n x 64j); ||z||^2 via per-group DVE
    square+reduce overlapped with conv4, then an fp32 matmul against a
    ones column; 1 + ||c||^2 is a host-precomputed (32,64) fp32 tile.
  - q = reciprocal(1+d2) row-normalized on DVE, DMA out as fp32.
  - PE pre-warm: dummy matmuls during the w1/x DMA lead-in so HAM
    un-throttles (1.2 -> 2.4 GHz) before real conv work arrives.

Measured (8 axon trn2 cores): fp8 HW probe rel err ~1.6e-4 per matmul;
fp16 matmuls hard-fault the device (NRT_EXEC_UNIT_UNRECOVERABLE) - do not
use fp16. fp8e4 (this file) verified working via DoubleRow probes.
"""

import os
import sys

import numpy as np
import ml_dtypes

for _p in ("/opt/trn_rl_repo",):
    if _p not in sys.path and os.path.isdir(_p):
        sys.path.insert(0, _p)

import concourse.bacc as bacc  # noqa: E402
import concourse.mybir as mybir  # noqa: E402
import concourse.tile as tile  # noqa: E402
from concourse import bass_utils  # noqa: E402

F8 = mybir.dt.float8e4
HDT = mybir.dt.bfloat16
F32 = mybir.dt.float32
AF = mybir.ActivationFunctionType
OP = mybir.AluOpType
DR = mybir.MatmulPerfMode.DoubleRow

N_CORES = 8
NS = 32          # samples per core
C = 128          # channels
KCENT = 64       # number of centers
LFIN = 59        # final length
D = C * LFIN     # 7552

# (K_real, K_padded, L_in_row_stride, L_out, L_out_row_stride, G samples/mm)
# conv2's output rows carry 1 pad element (248) so conv3's zero tap 7 reads
# in-bounds; the pad is memset once.
CFG = [
    (15, 16, 1024, 505, 505, 1),
    (12, 12, 505, 247, 248, 2),
    (7, 8, 248, 121, 121, 4),
    (4, 4, 121, 59, 59, 8),
]

N_WARM = 44  # PE pre-warm dummy matmuls

_BUILt = {}


def _build_program(n_repeat=1):
    """Build + compile the per-core Bass program (same program on all cores).

    n_repeat > 1 unrolls the full per-inference body that many times inside
    one NEFF (constants loaded once) — used only for slope timing in bench.
    """
    nc = bacc.Bacc("TRN2", target_bir_lowering=False, debug=False)

    # ---- DRAM I/O ----
    x_d = nc.dram_tensor("x", (C, NS, 1024), F8, kind="ExternalInput")
    w_d = [
        nc.dram_tensor(f"w{i+1}", (C, CFG[i][1] * C), F8, kind="ExternalInput")
        for i in range(4)
    ]
    # bias/scale pack: cols 0-3 = b1..b4; cols 4-7 = 2^-k1..2^-k4;
    # cols 8-71 = ones (zn mm rhs); cols 72-135 = row0-only 1+||c||^2
    # (cn mm rhs); cols 136-167 = e0 (partition-0 ones, cn mm lhsT)
    bp_d = nc.dram_tensor("bp", (C, 168), F32, kind="ExternalInput")
    # fp8 centers: cr8[c, l*64+j] = (-2/16)*centers[j, c*59+l], chunk 59 = 0
    cr_d = nc.dram_tensor("cr", (C, 60 * KCENT), F8, kind="ExternalInput")
    q_d = nc.dram_tensor("q", (NS, KCENT), F32, kind="ExternalOutput")

    with tile.TileContext(nc) as tc:
        with (
            tc.tile_pool(name="consts", bufs=1) as cpool,
            tc.tile_pool(name="xp", bufs=8) as xpool,
            tc.tile_pool(name="hp", bufs=1) as hpool,
            tc.tile_pool(name="ap", bufs=3) as apool,
            tc.tile_pool(name="tp", bufs=3) as tpool,
            tc.tile_pool(name="small", bufs=1) as mpool,
            tc.tile_pool(name="psA", bufs=3, space="PSUM") as psA,
            tc.tile_pool(name="psD", bufs=2, space="PSUM") as psD,
        ):
            wt = [
                cpool.tile([C, CFG[i][1] * C], F8, tag=f"w{i}", name=f"wt{i}")
                for i in range(4)
            ]
            bp = cpool.tile([C, 168], F32, tag="bp")
            cr = cpool.tile([C, 60 * KCENT], F8, tag="cr")

            for _rep in range(n_repeat):
                _body_once(nc, tc, x_d, q_d, w_d, bp_d, cr_d, wt, bp,
                           cr, xpool, hpool, apool, tpool, mpool,
                           psA, psD, load_consts=(_rep == 0))

    nc.compile()
    return nc


def _body_once(nc, tc, x_d, q_d, w_d, bp_d, cr_d, wt, bp, cr,
               xpool, hpool, apool, tpool, mpool, psA, psD,
               load_consts=True):
            # ---- Two HWDGE rings: x chunks stream on the SP ring while all
            # constants go on the ACT ring, so w1 arrives concurrently with
            # x0 and conv1 starts early ----
            if load_consts:
                nc.scalar.dma_start(wt[0][:], w_d[0].ap())
                nc.scalar.dma_start(bp[:], bp_d.ap())
            xch = []
            for g in range(16):
                t = xpool.tile([C, 2 * 1024], F8, tag="x", name=f"xch{g}")
                src = x_d.ap()[:, 2 * g : 2 * g + 2, :].rearrange("p a b -> p (a b)")
                nc.sync.dma_start(t[:], src)
                xch.append(t)
            if load_consts:
                for i in range(1, 4):
                    nc.scalar.dma_start(wt[i][:], w_d[i].ap())
                nc.scalar.dma_start(cr[:], cr_d.ap())

                # ---- PE pre-warm: HAM un-throttles (1.2 -> 2.4 GHz) after
                # ~3us of sustained activity; burn the w1/x0 DMA lead-in on
                # dummy matmuls over a zeroed scratch so conv1 starts warm ----
                wsrc = tpool.tile([1, 128], HDT, tag="warm", name="warm")
                nc.gpsimd.memset(wsrc[:], 0.0)
                wps = psA.tile([C, 128], F32, tag="ps", name="warmps")
                for _w in range(N_WARM):
                    nc.tensor.matmul(
                        wps[:], wsrc[:], wsrc[:],
                        start=(_w == 0), stop=(_w == N_WARM - 1)
                    )

            # ---- conv stack (fp8 DoubleRow tap-pair matmuls) ----
            h_tiles = []
            for li, (K, Kp, Lin, Lout, Lrow, G) in enumerate(CFG):
                if li == 3:
                    hdst = hpool.tile([C, NS * Lout], HDT, tag=f"h{li}")
                else:
                    hdst = hpool.tile([C, NS * Lrow], F8, tag=f"h{li}")
                    if Lrow > Lout:
                        # zero the per-sample pad so the zero tap's
                        # in-bounds read never multiplies NaN garbage
                        padv = hdst[:].rearrange("p (n l) -> p n l", n=NS)
                        nc.gpsimd.memset(padv[:, :, Lout:Lrow], 0.0)
                if li > 0:
                    hsrc3 = h_tiles[li - 1][:].rearrange(
                        "p (n l) -> p n l", n=NS
                    )
                hdst3 = hdst[:].rearrange("p (n l) -> p n l", n=NS)
                # Two G-sample groups share one 2-bank PSUM tile (bank-aligned
                # halves) so each eviction pass covers both in one instruction
                for gp in range(0, NS, 2 * G):
                    ps = psA.tile([C, 1024], F32, tag="ps")
                    for half in range(2):
                        g0 = gp + half * G
                        pslice = ps[:, half * 512 : half * 512 + G * Lout]
                        for kp in range(0, Kp, 2):
                            lhsT = wt[li][:, kp * C : (kp + 2) * C].rearrange(
                                "p (two o) -> p two o", two=2
                            )
                            if li == 0:
                                x3 = xch[g0 // 2][:].rearrange(
                                    "p (a b) -> p a b", a=2
                                )
                                rhs = x3[
                                    :, g0 % 2 : g0 % 2 + 1, kp : kp + 2 * Lout
                                ].rearrange("p n (l two) -> p two n l", two=2)
                            else:
                                rhs = hsrc3[
                                    :, g0 : g0 + G, kp : kp + 2 * Lout
                                ].rearrange("p n (l two) -> p two n l", two=2)
                            nc.tensor.matmul(
                                pslice, lhsT, rhs,
                                start=(kp == 0), stop=(kp == Kp - 2),
                                perf_mode=DR,
                            )
                    bias = bp[:, li : li + 1]
                    scale = bp[:, 4 + li : 5 + li]
                    # conv1-3: evict both halves in one pass (throughput);
                    # conv4: evict per half (latency - feeds the per-pair
                    # distance stage sooner)
                    halves = (
                        [(gp, ps[:].rearrange("p (g l) -> p g l", g=2)[:, :, 0 : G * Lout], 2 * G)]
                        if li < 3
                        else [
                            (gp, ps[:, 0 : G * Lout], G),
                            (gp + G, ps[:, 512 : 512 + G * Lout], G),
                        ]
                    )
                    for e0, psv, ng in halves:
                        E = ng * Lout
                        if li < 3:
                            # LeakyReLU split across engines:
                            #   ACT: A = ps*2^-k + b   Pool/DVE: t = 0.1*A
                            #   DVE: h = max(A, t) -> fp8
                            A = apool.tile([C, E], HDT, tag="A")
                            nc.scalar.activation(
                                A[:], psv, AF.Identity, bias=bias, scale=scale
                            )
                            t = tpool.tile([C, E], HDT, tag="t")
                            ts_eng = nc.gpsimd if li < 1 else nc.vector
                            ts_eng.tensor_scalar_mul(t[:], A[:], 0.1)
                            dsl = hdst3[:, e0 : e0 + ng, 0:Lout]
                            A3 = A[:].rearrange("p (n l) -> p n l", n=ng)
                            t3 = t[:].rearrange("p (n l) -> p n l", n=ng)
                            nc.vector.tensor_tensor(dsl, A3, t3, op=OP.max)
                        else:
                            dsl = hdst[:, e0 * Lout : (e0 + ng) * Lout]
                            nc.scalar.activation(
                                dsl, psv, AF.Identity, bias=bias, scale=scale
                            )
                            # per-half: ||z||^2 partials + fp8 z8 cast
                            # (position-major, x16) for the DR distance
                            if e0 == 0:
                                zsq = hpool.tile(
                                    [C, NS * LFIN], F32, tag="zsq", name="zsq"
                                )
                                part = mpool.tile(
                                    [C, NS], F32, tag="part", name="part"
                                )
                                z8 = hpool.tile(
                                    [C, 60 * NS], F8, tag="z8", name="z8"
                                )
                                nc.gpsimd.memset(
                                    z8[:, LFIN * NS : 60 * NS], 0.0
                                )
                            zsl = zsq[:, e0 * LFIN : (e0 + ng) * LFIN]
                            nc.vector.tensor_tensor(zsl, dsl, dsl, op=OP.mult)
                            nc.vector.tensor_reduce(
                                part[:, e0 : e0 + ng],
                                zsl.rearrange("p (n l) -> p n l", n=ng),
                                axis=mybir.AxisListType.X,
                                op=OP.add,
                            )
                            z83 = z8[:].rearrange("p (l n) -> p l n", n=NS)
                            dsl3 = hdst[
                                :, e0 * Lout : (e0 + ng) * Lout
                            ].rearrange("p (n l) -> p n l", n=ng)
                            nc.vector.tensor_scalar_mul(
                                z83[:, 0:LFIN, e0 : e0 + ng].rearrange(
                                    "p l n -> p n l"
                                ),
                                dsl3,
                                16.0,
                            )
                h_tiles.append(hdst)

            # ---- d2 = -2 z.c + ||z||^2 + (1 + ||c||^2), all in PSUM,
            # one 16-sample conv4 pair per PSUM bank (partition base 0).
            # Group order: cn (start) -> 30 fp8-DR dist pair-chunks -> zn
            # (stop); cn/zn are fp32 matmuls (e0 x cnrow, part x ones). ----
            z84 = z8[:].rearrange("p (l n) -> p l n", n=NS)
            cr3 = cr[:].rearrange("p (l j) -> p l j", j=KCENT)
            for p in range(2):
                dp = psD.tile([16, KCENT], F32, tag="d")
                nc.tensor.matmul(
                    dp[:], bp[:, 136:152], bp[:, 72:136],
                    start=True, stop=False,
                )
                for lp in range(0, 60, 2):
                    lhsT = z84[:, lp : lp + 2, 16 * p : 16 * p + 16]
                    nc.tensor.matmul(
                        dp[:], lhsT, cr3[:, lp : lp + 2, :],
                        start=False, stop=False, perf_mode=DR,
                    )
                nc.tensor.matmul(
                    dp[:], part[:, 16 * p : 16 * p + 16], bp[:, 8:72],
                    start=False, stop=True,
                )

                # ---- q = normalize(1/d2') for this 16-sample half ----
                qn = mpool.tile([16, KCENT], F32, tag=f"qn{p}")
                nc.vector.reciprocal(qn[:], dp[:])
                rs = mpool.tile([16, 1], F32, tag=f"rs{p}")
                nc.vector.tensor_reduce(
                    rs[:], qn[:], axis=mybir.AxisListType.X, op=OP.add
                )
                rr = mpool.tile([16, 1], F32, tag=f"rr{p}")
                nc.vector.reciprocal(rr[:], rs[:])
                nc.vector.tensor_scalar_mul(qn[:], qn[:], rr[:])
                nc.sync.dma_start(q_d.ap()[16 * p : 16 * p + 16, :], qn[:])


def _get_program(n_repeat=1):
    if n_repeat not in _BUILt:
        _BUILt[n_repeat] = _build_program(n_repeat)
    return _BUILt[n_repeat]


def _to_f8(a):
    """fp32 -> TRN E4M3 (max 240; clip so OCP e4m3fn bit patterns match)."""
    return np.clip(a, -240.0, 240.0).astype(ml_dtypes.float8_e4m3fn)


def _prep_inputs(x, w1, b1, w2, b2, w3, b3, w4, b4, centers):
    """Host-side prep: fp8 quantization, weight transposes, sharding."""
    ws = [w1, w2, w3, w4]
    bs = [b1, b2, b3, b4]

    const_map = {}
    scales = []
    for i, w in enumerate(ws):
        K, Kp = CFG[i][0], CFG[i][1]
        wf = np.asarray(w, np.float32)  # (O, I, K)
        # per-layer power-of-2 scale-up to ~224 max magnitude (e4m3 headroom)
        mx = float(np.abs(wf).max())
        k = int(np.floor(np.log2(224.0 / mx))) if mx > 0 else 0
        scales.append(2.0 ** (-k))
        wq = wf * (2.0 ** k)
        # (O, I, K) -> (I, Kp, O): lhsT tap k = [:, k*128:(k+1)*128]
        wp = np.zeros((C, Kp, C), np.float32)
        wp[:, :K, :] = wq.transpose(1, 2, 0)
        const_map[f"w{i+1}"] = _to_f8(wp.reshape(C, Kp * C))

    cent = np.asarray(centers, np.float32)
    # cr8[c, l*64 + j] = (-2/16) * centers[j, c*59 + l]; position chunk 59
    # is zero (pairs the z8 pad so the DR distance contracts 60 positions).
    # The 1/16 undoes z8's x16 pre-scale (both powers of 2, exact).
    cr8 = np.zeros((C, 60, KCENT), np.float32)
    cr8[:, :LFIN, :] = (
        (-2.0 / 16.0 * cent).reshape(KCENT, C, LFIN).transpose(1, 2, 0)
    )
    const_map["cr"] = _to_f8(cr8.reshape(C, 60 * KCENT))
    cn = 1.0 + (cent.astype(np.float64) ** 2).sum(axis=1)  # (64,)

    bpk = np.zeros((C, 168), np.float32)
    for i, b in enumerate(bs):
        bpk[:, i] = np.asarray(b, np.float32)
        bpk[:, 4 + i] = scales[i]
    bpk[:, 8:72] = 1.0                      # zn mm rhs (ones)
    bpk[0, 72:136] = cn.astype(np.float32)  # cn mm rhs (row 0 only)
    bpk[0, 136:168] = 1.0                   # cn mm lhsT e0 (row 0 only)
    const_map["bp"] = bpk

    xf = np.asarray(x, np.float32)
    in_maps = []
    for c in range(N_CORES):
        shard = xf[c * NS : (c + 1) * NS]  # (32, 128, 1024)
        xc = _to_f8(np.ascontiguousarray(shard.transpose(1, 0, 2)))  # (128,32,1024)
        in_maps.append({"x": xc, **const_map})
    return in_maps


def _ensure_devices():
    """Absorb wedged-device attach faults with a tiny op before the real run.

    A previous process can leave a NeuronCore wedged
    (NRT_EXEC_UNIT_UNRECOVERABLE); the first attach after a wedge fails and
    triggers a reset that completes within ~60 s.
    """
    import time

    import jax
    import jax.numpy as jnp

    for attempt in range(3):
        try:
            outs = [jax.device_put(jnp.zeros((8,)), d) + 1.0 for d in jax.devices()]
            jax.block_until_ready(outs)
            return
        except Exception:  # noqa: BLE001 - device fault; wait out the reset
            if attempt == 2:
                raise
            time.sleep(60)


def run(trace=False, **inputs):
    """Run the kernel; returns (q_full, BassKernelResults).

    Retries on device-unrecoverable faults (see _ensure_devices).
    """
    import time

    _ensure_devices()
    nc = _get_program()
    in_maps = _prep_inputs(**inputs)
    last_err = None
    for attempt in range(3):
        try:
            res = bass_utils.run_bass_kernel_spmd(
                nc, in_maps, core_ids=list(range(N_CORES)), trace=trace
            )
            break
        except Exception as e:  # noqa: BLE001 - device fault, wait + retry
            last_err = e
            if "UNAVAILABLE" not in str(e) and "unrecoverable" not in str(e).lower():
                raise
            time.sleep(60)
    else:
        raise last_err
    q = np.concatenate([res.results[c]["q"] for c in range(N_CORES)], axis=0)
    return np.ascontiguousarray(q.astype(np.float32)), res


def kernel(**inputs) -> np.ndarray:
    q, _ = run(trace=False, **inputs)
    return q


# revision 57
# speedup vs baseline: 1505.6551x; 1.0017x over previous
"""Trainium2 Bass kernel for nn_DEC_26139170963600 (vq_codebook).

Reference computation:
  4x strided conv1d (stride 2, VALID) with LeakyReLU(0.1) between layers,
  flatten -> soft VQ assignment over 64 centers:
      d2 = ||z||^2 + ||c||^2 - 2 z.c
      q  = (1/(1+d2)) row-normalized            (alpha=1 -> exponent is 1)

Sharding: data-parallel over batch N=256 across 8 cores (32 samples/core).
Weights / centers replicated. No cross-device communication.

Per-core kernel design (fp8 DoubleRow conv stack):
  - x and all conv weights quantized host-side to fp8e4 (TRN E4M3, max 240).
    Weights are pre-scaled by a per-layer power of two (up to ~224 max mag)
    to clear e4m3 subnormals; the inverse scale rides the PSUM eviction.
  - conv layer = K/2 tap-pair matmuls in MatmulPerfMode.DoubleRow, which
    virtualizes the PE to a 256-deep contraction (2 fp8 weights/cell):
        out[o, l] += sum_i W[o,:,k+i]^T . h[:, 2l+k+i],  i in {0,1}
    lhsT = (128, 2, 128) tap-pair slice of the packed weights; rhs is the
    contiguous slice h[k : k+2*Lout] rearranged "(l two) -> two l" (the
    stride-2 conv makes tap pairs adjacent in memory). Odd K padded with a
    zero tap (conv1 15->16, conv3 7->8; h2 rows are stride-248-padded so
    the zero tap's read stays in-bounds).
  - PSUM eviction splits LeakyReLU over three engines so no engine exceeds
    the PE's busy time:  ACT: A = ps*2^-k + b (PSUM->SBUF bf16);
    Pool (conv1) / DVE (conv2-3): t = 0.1*A;  DVE: h' = max(A, t) -> fp8
    (exact lrelu since 0.1 < 1). Two G-sample groups share a 2-bank PSUM
    tile so each eviction pass covers both in one instruction. Conv blocks
    are emitted in a four-stream interleaved order so eviction latency
    hides under another stream's PE work.
  - CAUTION (probe-verified): ACT Prelu computes exact alpha*x and would
    fuse the whole eviction into one op (sims 56 us), but it hard-faults
    the device when >=4 cores run it concurrently. Same for moving conv2's
    tensor_scalar onto Pool. Both are disabled; see EVICT_PRELU.
  - conv4 evicts bf16 z (no lrelu); distance stays bf16: 59 matmuls
    accumulate -2 z.c into PSUM (32n x 64j); ||z||^2 via per-group DVE
    square+reduce overlapped with conv4, then an fp32 matmul against a
    ones column; 1 + ||c||^2 is a host-precomputed (32,64) fp32 tile.
  - q = reciprocal(1+d2) row-normalized on DVE, DMA out as fp32.
  - PE pre-warm: dummy matmuls during the w1/x DMA lead-in so HAM
    un-throttles (1.2 -> 2.4 GHz) before real conv work arrives.

Measured (8 axon trn2 cores): max rel err 2.34e-3 vs fp32 reference;
TimelineSim 63.2 us (baseline bf16 kernel: 168.4 us). fp16 matmuls
hard-fault the device (NRT_EXEC_UNIT_UNRECOVERABLE) - do not use fp16.
"""

import os
import sys

import numpy as np
import ml_dtypes

for _p in ("/opt/trn_rl_repo",):
    if _p not in sys.path and os.path.isdir(_p):
        sys.path.insert(0, _p)

import concourse.bacc as bacc  # noqa: E402
import concourse.mybir as mybir  # noqa: E402
import concourse.tile as tile  # noqa: E402
from concourse import bass_utils  # noqa: E402

F8 = mybir.dt.float8e4
HDT = mybir.dt.bfloat16
F32 = mybir.dt.float32
AF = mybir.ActivationFunctionType
OP = mybir.AluOpType
DR = mybir.MatmulPerfMode.DoubleRow

N_CORES = 8
NS = 32          # samples per core
C = 128          # channels
KCENT = 64       # number of centers
LFIN = 59        # final length
D = C * LFIN     # 7552

# (K_real, K_padded, L_in_row_stride, L_out, L_out_row_stride, G samples/mm)
# conv2's output rows carry 1 pad element (248) so conv3's zero tap 7 reads
# in-bounds; the pad is memset once.
CFG = [
    (15, 16, 1024, 505, 505, 1),
    (12, 12, 505, 247, 248, 2),
    (7, 8, 248, 121, 121, 4),
    (4, 4, 121, 59, 59, 8),
]

N_WARM = 44  # PE pre-warm dummy matmuls

INTERLEAVE = True  # four-stream schedule vs plain layer order
# Single-pass ACT Prelu eviction sims ~4us faster but hard-faults the
# device when >=4 cores run concurrently (NRT unrecoverable; 1-2 cores
# fine, probe-verified correct). Keep the 3-op max form on hardware.
EVICT_PRELU = False


def _schedule():
    """Block emission order. Interleaved: four 8-sample conv streams s0-s3
    (conv1 blocks = 2 samples, conv2 = 4, conv3/4 = 8) and two 16-sample
    distance regions; each stage's eviction latency hides under another
    stream's PE work."""
    if not INTERLEAVE:
        return (
            [("c", 0, p) for p in range(16)]
            + [("c", 1, p) for p in range(8)]
            + [("c", 2, p) for p in range(4)]
            + [("c", 3, p) for p in range(4)]
            + [("d", 0), ("q", 0), ("d", 1), ("q", 1)]
        )
    return [
        ("c", 0, 0), ("c", 0, 1), ("c", 0, 2), ("c", 0, 3),
        ("c", 0, 4), ("c", 0, 5), ("c", 0, 6), ("c", 0, 7),
        ("c", 1, 0), ("c", 1, 1),
        ("c", 0, 8), ("c", 0, 9), ("c", 0, 10), ("c", 0, 11),
        ("c", 1, 2), ("c", 1, 3),
        ("c", 2, 0),
        ("c", 0, 12), ("c", 0, 13), ("c", 0, 14), ("c", 0, 15),
        ("c", 1, 4), ("c", 1, 5),
        ("c", 3, 0),
        ("c", 2, 1),
        ("c", 1, 6), ("c", 1, 7),
        ("c", 3, 1),
        ("c", 2, 2),
        ("d", 0),
        ("c", 2, 3),
        ("c", 3, 2),
        ("q", 0),
        ("c", 3, 3),
        ("d", 1),
        ("q", 1),
    ]

_BUILt = {}


def _build_program(n_repeat=1):
    """Build + compile the per-core Bass program (same program on all cores).

    n_repeat > 1 unrolls the full per-inference body that many times inside
    one NEFF (constants loaded once) — used only for slope timing in bench.
    """
    nc = bacc.Bacc("TRN2", target_bir_lowering=False, debug=False)

    # ---- DRAM I/O ----
    x_d = nc.dram_tensor("x", (C, NS, 1024), F8, kind="ExternalInput")
    w_d = [
        nc.dram_tensor(f"w{i+1}", (C, CFG[i][1] * C), F8, kind="ExternalInput")
        for i in range(4)
    ]
    # bias/scale pack: cols 0-3 = b1..b4; cols 4-7 = 2^-k1..2^-k4;
    # cols 8-71 = ones (zn mm rhs); cols 72-135 = row0-only 1+||c||^2
    # (cn mm rhs); cols 136-167 = e0 (partition-0 ones, cn mm lhsT)
    bp_d = nc.dram_tensor("bp", (C, 168), F32, kind="ExternalInput")
    # fp8 centers: cr8[c, l*64+j] = (-2/16)*centers[j, c*59+l], chunk 59 = 0
    cr_d = nc.dram_tensor("cr", (C, 60 * KCENT), F8, kind="ExternalInput")
    q_d = nc.dram_tensor("q", (NS, KCENT), F32, kind="ExternalOutput")

    with tile.TileContext(nc) as tc:
        with (
            tc.tile_pool(name="consts", bufs=1) as cpool,
            tc.tile_pool(name="xp", bufs=8) as xpool,
            tc.tile_pool(name="hp", bufs=1) as hpool,
            tc.tile_pool(name="ap", bufs=4) as apool,
            tc.tile_pool(name="tp", bufs=4) as tpool,
            tc.tile_pool(name="small", bufs=1) as mpool,
            tc.tile_pool(name="psA", bufs=3, space="PSUM") as psA,
            tc.tile_pool(name="psD", bufs=2, space="PSUM") as psD,
        ):
            wt = [
                cpool.tile([C, CFG[i][1] * C], F8, tag=f"w{i}", name=f"wt{i}")
                for i in range(4)
            ]
            bp = cpool.tile([C, 168], F32, tag="bp")
            cr = cpool.tile([C, 60 * KCENT], F8, tag="cr")

            for _rep in range(n_repeat):
                _body_once(nc, tc, x_d, q_d, w_d, bp_d, cr_d, wt, bp,
                           cr, xpool, hpool, apool, tpool, mpool,
                           psA, psD, load_consts=(_rep == 0))

    nc.compile()
    return nc


def _body_once(nc, tc, x_d, q_d, w_d, bp_d, cr_d, wt, bp, cr,
               xpool, hpool, apool, tpool, mpool, psA, psD,
               load_consts=True):
            # ---- Two HWDGE rings: x chunks stream on the SP ring while all
            # constants go on the ACT ring, so w1 arrives concurrently with
            # x0 and conv1 starts early ----
            if load_consts:
                nc.scalar.dma_start(wt[0][:], w_d[0].ap())
                nc.scalar.dma_start(bp[:], bp_d.ap())
            xch = []
            for g in range(16):
                t = xpool.tile([C, 2 * 1024], F8, tag="x", name=f"xch{g}")
                src = x_d.ap()[:, 2 * g : 2 * g + 2, :].rearrange("p a b -> p (a b)")
                nc.sync.dma_start(t[:], src)
                xch.append(t)
            if load_consts:
                for i in range(1, 4):
                    nc.scalar.dma_start(wt[i][:], w_d[i].ap())
                nc.scalar.dma_start(cr[:], cr_d.ap())

                # ---- PE pre-warm: HAM un-throttles (1.2 -> 2.4 GHz) after
                # ~3us of sustained activity; burn the w1/x0 DMA lead-in on
                # dummy matmuls over a zeroed scratch so conv1 starts warm ----
                wsrc = tpool.tile([1, 128], HDT, tag="warm", name="warm")
                nc.gpsimd.memset(wsrc[:], 0.0)
                wps = psA.tile([C, 128], F32, tag="ps", name="warmps")
                for _w in range(N_WARM):
                    nc.tensor.matmul(
                        wps[:], wsrc[:], wsrc[:],
                        start=(_w == 0), stop=(_w == N_WARM - 1)
                    )

            # ---- shared tiles for the conv stack / distance tail ----
            h_tiles = []
            for li, (K, Kp, Lin, Lout, Lrow, G) in enumerate(CFG):
                if li == 3:
                    hdst = hpool.tile([C, NS * Lout], HDT, tag=f"h{li}")
                else:
                    hdst = hpool.tile([C, NS * Lrow], F8, tag=f"h{li}")
                    if Lrow > Lout:
                        # zero the per-sample pad so the zero tap's
                        # in-bounds read never multiplies NaN garbage
                        padv = hdst[:].rearrange("p (n l) -> p n l", n=NS)
                        nc.gpsimd.memset(padv[:, :, Lout:Lrow], 0.0)
                h_tiles.append(hdst)
            zsq = hpool.tile([C, NS * LFIN], F32, tag="zsq", name="zsq")
            part = mpool.tile([C, NS], F32, tag="part", name="part")
            z8 = hpool.tile([C, 60 * NS], F8, tag="z8", name="z8")
            nc.gpsimd.memset(z8[:, LFIN * NS : 60 * NS], 0.0)
            z84 = z8[:].rearrange("p (l n) -> p l n", n=NS)
            cr3 = cr[:].rearrange("p (l j) -> p l j", j=KCENT)
            dps = [None] * 2

            def conv_block(li, pr):
                """One PSUM block: conv1-3 = two G-sample halves sharing a
                2-bank tile; conv4 = one 8-sample group on half a tile."""
                K, Kp, Lin, Lout, Lrow, G = CFG[li]
                hdst = h_tiles[li]
                hdst3 = hdst[:].rearrange("p (n l) -> p n l", n=NS)
                if li > 0:
                    hsrc3 = h_tiles[li - 1][:].rearrange("p (n l) -> p n l", n=NS)
                nhalf = 1 if li == 3 else 2
                gp = pr * nhalf * G
                ps = psA.tile([C, 1024], F32, tag="ps")
                for half in range(nhalf):
                    g0 = gp + half * G
                    pslice = ps[:, half * 512 : half * 512 + G * Lout]
                    for kp in range(0, Kp, 2):
                        lhsT = wt[li][:, kp * C : (kp + 2) * C].rearrange(
                            "p (two o) -> p two o", two=2
                        )
                        if li == 0:
                            x3 = xch[g0 // 2][:].rearrange("p (a b) -> p a b", a=2)
                            rhs = x3[
                                :, g0 % 2 : g0 % 2 + 1, kp : kp + 2 * Lout
                            ].rearrange("p n (l two) -> p two n l", two=2)
                        else:
                            rhs = hsrc3[
                                :, g0 : g0 + G, kp : kp + 2 * Lout
                            ].rearrange("p n (l two) -> p two n l", two=2)
                        nc.tensor.matmul(
                            pslice, lhsT, rhs,
                            start=(kp == 0), stop=(kp == Kp - 2),
                            perf_mode=DR,
                        )
                bias = bp[:, li : li + 1]
                scale = bp[:, 4 + li : 5 + li]
                ng = nhalf * G
                E = ng * Lout
                psv = (
                    ps[:, 0 : G * Lout]
                    if li == 3
                    else ps[:].rearrange("p (g l) -> p g l", g=2)[:, :, 0 : G * Lout]
                )
                if li < 3:
                    dsl = hdst3[:, gp : gp + ng, 0:Lout]
                    if EVICT_PRELU:
                        # LeakyReLU in ONE ACT pass straight to fp8:
                        # h = Prelu(ps*2^-k + b, alpha=0.1). (HW Prelu
                        # honors alpha exactly, unlike Lrelu whose table
                        # hardcodes 0.01 - probe-verified.)
                        nc.scalar.activation(
                            dsl, psv, AF.Prelu, bias=bias, scale=scale,
                            alpha=0.1,
                        )
                    else:
                        # 3-op fallback: ACT: A = ps*2^-k + b;
                        # Pool/DVE: t = 0.1A; DVE: h = max(A, t) -> fp8
                        A = apool.tile([C, E], HDT, tag="A")
                        nc.scalar.activation(
                            A[:], psv, AF.Identity, bias=bias, scale=scale
                        )
                        t = tpool.tile([C, E], HDT, tag="t")
                        ts_eng = nc.gpsimd if li < 1 else nc.vector
                        ts_eng.tensor_scalar_mul(t[:], A[:], 0.1)
                        A3 = A[:].rearrange("p (n l) -> p n l", n=ng)
                        t3 = t[:].rearrange("p (n l) -> p n l", n=ng)
                        nc.vector.tensor_tensor(dsl, A3, t3, op=OP.max)
                else:
                    # conv4: bf16 z eviction + ||z||^2 partials + fp8 z8
                    # cast (position-major, x16) for the DR distance.
                    # Engines spread to keep DVE off the tail critical path:
                    # ACT squares, Pool reduces, DVE only casts z8.
                    dsl = hdst[:, gp * Lout : (gp + ng) * Lout]
                    if EVICT_PRELU:
                        # Prelu(alpha=1) == Identity; keeps every ACT op on
                        # the Prelu table (no mid-kernel table switches)
                        nc.scalar.activation(
                            dsl, psv, AF.Prelu, bias=bias, scale=scale,
                            alpha=1.0,
                        )
                    else:
                        nc.scalar.activation(
                            dsl, psv, AF.Identity, bias=bias, scale=scale
                        )
                    zsl = zsq[:, gp * LFIN : (gp + ng) * LFIN]
                    nc.vector.tensor_tensor(zsl, dsl, dsl, op=OP.mult)
                    nc.vector.tensor_reduce(
                        part[:, gp : gp + ng],
                        zsl.rearrange("p (n l) -> p n l", n=ng),
                        axis=mybir.AxisListType.X,
                        op=OP.add,
                    )
                    z83 = z8[:].rearrange("p (l n) -> p l n", n=NS)
                    dsl3 = dsl.rearrange("p (n l) -> p n l", n=ng)
                    nc.vector.tensor_scalar_mul(
                        z83[:, 0:LFIN, gp : gp + ng].rearrange("p l n -> p n l"),
                        dsl3,
                        16.0,
                    )

            def dist_block(p):
                """d2 for 16 samples in one PSUM bank (partition base 0):
                cn (start) -> 30 fp8-DR position-pair chunks -> zn (stop);
                cn/zn are fp32 matmuls (e0 x cnrow, part x ones)."""
                dp = psD.tile([16, KCENT], F32, tag="d")
                dps[p] = dp
                nc.tensor.matmul(
                    dp[:], bp[:, 136:152], bp[:, 72:136],
                    start=True, stop=False,
                )
                for lp in range(0, 60, 2):
                    lhsT = z84[:, lp : lp + 2, 16 * p : 16 * p + 16]
                    nc.tensor.matmul(
                        dp[:], lhsT, cr3[:, lp : lp + 2, :],
                        start=False, stop=False, perf_mode=DR,
                    )
                nc.tensor.matmul(
                    dp[:], part[:, 16 * p : 16 * p + 16], bp[:, 8:72],
                    start=False, stop=True,
                )

            def q_block(p):
                """q = normalize(1/d2') for 16 samples; DMA out on the Pool
                ring (keeps the SP ring's serial dispatch off the tail)."""
                dp = dps[p]
                qn = mpool.tile([16, KCENT], F32, tag=f"qn{p}")
                nc.vector.reciprocal(qn[:], dp[:])
                rs = mpool.tile([16, 1], F32, tag=f"rs{p}")
                nc.vector.tensor_reduce(
                    rs[:], qn[:], axis=mybir.AxisListType.X, op=OP.add
                )
                rr = mpool.tile([16, 1], F32, tag=f"rr{p}")
                nc.vector.reciprocal(rr[:], rs[:])
                nc.vector.tensor_scalar_mul(qn[:], qn[:], rr[:])
                nc.sync.dma_start(q_d.ap()[16 * p : 16 * p + 16, :], qn[:])

            # ---- interleaved schedule: four 8-sample conv streams s0-s3
            # (conv1 blocks = 2 samples, conv2 = 4, conv3/4 = 8) and two
            # 16-sample distance regions; each stage's eviction latency
            # hides under another stream's PE work ----
            for blk in _schedule():
                if blk[0] == "c":
                    conv_block(blk[1], blk[2])
                elif blk[0] == "d":
                    dist_block(blk[1])
                else:
                    q_block(blk[1])


def _get_program(n_repeat=1):
    if n_repeat not in _BUILt:
        _BUILt[n_repeat] = _build_program(n_repeat)
    return _BUILt[n_repeat]


def _to_f8(a):
    """fp32 -> TRN E4M3 (max 240; clip so OCP e4m3fn bit patterns match)."""
    return np.clip(a, -240.0, 240.0).astype(ml_dtypes.float8_e4m3fn)


def _prep_inputs(x, w1, b1, w2, b2, w3, b3, w4, b4, centers):
    """Host-side prep: fp8 quantization, weight transposes, sharding."""
    ws = [w1, w2, w3, w4]
    bs = [b1, b2, b3, b4]

    const_map = {}
    scales = []
    for i, w in enumerate(ws):
        K, Kp = CFG[i][0], CFG[i][1]
        wf = np.asarray(w, np.float32)  # (O, I, K)
        # per-layer power-of-2 scale-up to ~224 max magnitude (e4m3 headroom)
        mx = float(np.abs(wf).max())
        k = int(np.floor(np.log2(224.0 / mx))) if mx > 0 else 0
        scales.append(2.0 ** (-k))
        wq = wf * (2.0 ** k)
        # (O, I, K) -> (I, Kp, O): lhsT tap k = [:, k*128:(k+1)*128]
        wp = np.zeros((C, Kp, C), np.float32)
        wp[:, :K, :] = wq.transpose(1, 2, 0)
        const_map[f"w{i+1}"] = _to_f8(wp.reshape(C, Kp * C))

    cent = np.asarray(centers, np.float32)
    # cr8[c, l*64 + j] = (-2/16) * centers[j, c*59 + l]; position chunk 59
    # is zero (pairs the z8 pad so the DR distance contracts 60 positions).
    # The 1/16 undoes z8's x16 pre-scale (both powers of 2, exact).
    cr8 = np.zeros((C, 60, KCENT), np.float32)
    cr8[:, :LFIN, :] = (
        (-2.0 / 16.0 * cent).reshape(KCENT, C, LFIN).transpose(1, 2, 0)
    )
    const_map["cr"] = _to_f8(cr8.reshape(C, 60 * KCENT))
    cn = 1.0 + (cent.astype(np.float64) ** 2).sum(axis=1)  # (64,)

    bpk = np.zeros((C, 168), np.float32)
    for i, b in enumerate(bs):
        bpk[:, i] = np.asarray(b, np.float32)
        bpk[:, 4 + i] = scales[i]
    bpk[:, 8:72] = 1.0                      # zn mm rhs (ones)
    bpk[0, 72:136] = cn.astype(np.float32)  # cn mm rhs (row 0 only)
    bpk[0, 136:168] = 1.0                   # cn mm lhsT e0 (row 0 only)
    const_map["bp"] = bpk

    xf = np.asarray(x, np.float32)
    in_maps = []
    for c in range(N_CORES):
        shard = xf[c * NS : (c + 1) * NS]  # (32, 128, 1024)
        xc = _to_f8(np.ascontiguousarray(shard.transpose(1, 0, 2)))  # (128,32,1024)
        in_maps.append({"x": xc, **const_map})
    return in_maps


def _ensure_devices():
    """Absorb wedged-device attach faults with a tiny op before the real run.

    A previous process can leave a NeuronCore wedged
    (NRT_EXEC_UNIT_UNRECOVERABLE); the first attach after a wedge fails and
    triggers a reset that completes within ~60 s.
    """
    import time

    import jax
    import jax.numpy as jnp

    for attempt in range(3):
        try:
            outs = [jax.device_put(jnp.zeros((8,)), d) + 1.0 for d in jax.devices()]
            jax.block_until_ready(outs)
            return
        except Exception:  # noqa: BLE001 - device fault; wait out the reset
            if attempt == 2:
                raise
            time.sleep(60)


def run(trace=False, **inputs):
    """Run the kernel; returns (q_full, BassKernelResults).

    Retries on device-unrecoverable faults (see _ensure_devices).
    """
    import time

    _ensure_devices()
    nc = _get_program()
    in_maps = _prep_inputs(**inputs)
    last_err = None
    for attempt in range(3):
        try:
            res = bass_utils.run_bass_kernel_spmd(
                nc, in_maps, core_ids=list(range(N_CORES)), trace=trace
            )
            break
        except Exception as e:  # noqa: BLE001 - device fault, wait + retry
            last_err = e
            if "UNAVAILABLE" not in str(e) and "unrecoverable" not in str(e).lower():
                raise
            time.sleep(60)
    else:
        raise last_err
    q = np.concatenate([res.results[c]["q"] for c in range(N_CORES)], axis=0)
    return np.ascontiguousarray(q.astype(np.float32)), res


def kernel(**inputs) -> np.ndarray:
    q, _ = run(trace=False, **inputs)
    return q
